# revision 1
# baseline (speedup 1.0000x reference)
"""Trainium2 Bass kernel for nn_BackgroundNoiseLayer.

Computation (see reference):
    spikes = (u < 0.25) as f32, shape (T=600, K=100)
    W = scatter_add(zeros(N=50000, K, R=5), (rows, cols), weights[:,None]*weights_factors)
    out[t, n, r] = sum_k W[n, k, r] * spikes[t, k]      -> (1, 600, 250000)

Sharding: postsynaptic neuron dim N is split across 8 NeuronCores (6250 rows
each).  Each core scatters its own (6250, 100, 5) W block (built on-device via
one-hot matmuls in PSUM, which accumulates duplicates exactly like the
reference scatter-add) and computes its (600, 6250, 5) output slice.

The one-hot factorization used for the scatter:
    W[k, (n,r)] += sum_e [cols_e == k] * weights_e  *  [rows_e == n] * factors_e[r]
so the stationary matmul operand is (cols one-hot * weights) and the moving
operand is (rows one-hot * factors), both built with a single dual-op
tensor_scalar (is_equal then mult) per tile.
"""

import sys

if "/opt/trn_rl_repo" not in sys.path:
    sys.path.insert(0, "/opt/trn_rl_repo")

import numpy as np

# ---- problem constants (hardcoded; kernel.py must be self-contained) ----
N_NEURONS = 50000
P_SPIKE = 0.25
N_CORES = 8
N_SHARD = N_NEURONS // N_CORES      # 6250
K = 100                             # background units
R = 5                               # syn basis
T_SEQ = 600                         # B*T
CHUNK_ROWS = 102                    # n rows per W chunk -> 510 free cols <= 512 (one PSUM bank)
N_CHUNKS = -(-N_SHARD // CHUNK_ROWS)  # 62 (61 full + 1 of 28 rows)
LAST_ROWS = N_SHARD - (N_CHUNKS - 1) * CHUNK_ROWS  # 28
TT = 5                              # t tiles
T_TILE = T_SEQ // TT                # 120
P = 128
DMA_GROUP = 8                       # chunks per output DMA

_CACHE = {}


def _build_nc(t_glob: int, reps: int = 1, mode: str = "full",
              use_f32r: bool = True, out_fp16: bool = True):
    """reps>1 wraps the main chunk loop in a device-side For loop — used only
    for benchmarking (wall-clock delta between rep counts isolates HW time).
    mode: 'full' | 'nodma' | 'dmaonly' | 'nobuild' | 'noscatter' (ablations
    for bottleneck bisection; only 'full' is used by kernel())."""
    import contextlib

    import concourse.bacc as bacc
    import concourse.tile as tile
    from concourse import mybir

    f32 = mybir.dt.float32
    mmdt = mybir.dt.float32r if use_f32r else f32
    odt = mybir.dt.float16 if out_fp16 else f32
    eq = mybir.AluOpType.is_equal
    mul = mybir.AluOpType.mult
    lt = mybir.AluOpType.is_lt

    n_tiles = N_CHUNKS * t_glob

    nc = bacc.Bacc("TRN2", target_bir_lowering=False, debug=False,
                   num_devices=N_CORES)

    uT = nc.dram_tensor("uT", [K, T_SEQ], f32, kind="ExternalInput")
    rrel = nc.dram_tensor("rrel", [P, n_tiles], f32, kind="ExternalInput")
    colf = nc.dram_tensor("colf", [P, n_tiles], f32, kind="ExternalInput")
    wts = nc.dram_tensor("wts", [P, n_tiles], f32, kind="ExternalInput")
    facs = nc.dram_tensor("facs", [P, n_tiles * R], f32, kind="ExternalInput")
    y = nc.dram_tensor("y", [T_SEQ, N_SHARD * R], odt,
                       kind="ExternalOutput")

    with tile.TileContext(nc) as tc:
        with (
            tc.tile_pool(name="const", bufs=1) as cpool,
            tc.tile_pool(name="edges", bufs=1) as epool,
            tc.tile_pool(name="build", bufs=4) as bpool,
            tc.tile_pool(name="wsb", bufs=3) as wpool,
            tc.tile_pool(name="osb", bufs=2) as opool,
            tc.tile_pool(name="psw", bufs=2, space="PSUM") as pswp,
            tc.tile_pool(name="pso", bufs=5, space="PSUM") as psop,
        ):
            # --- prolog: constants, edge data, spikes ---
            itab = cpool.tile([P, CHUNK_ROWS], mybir.dt.int32)
            nc.gpsimd.iota(itab[:, :], [[1, CHUNK_ROWS]], channel_multiplier=0)
            ntab = cpool.tile([P, CHUNK_ROWS], f32)
            nc.vector.tensor_copy(ntab[:, :], itab[:, :])

            rrel_sb = epool.tile([P, n_tiles], f32)
            colf_sb = epool.tile([P, n_tiles], f32)
            wts_sb = epool.tile([P, n_tiles], f32)
            facs_sb = epool.tile([P, n_tiles * R], f32)
            nc.sync.dma_start(rrel_sb[:, :], rrel[:, :])
            nc.sync.dma_start(colf_sb[:, :], colf[:, :])
            nc.sync.dma_start(wts_sb[:, :], wts[:, :])
            nc.sync.dma_start(facs_sb[:, :], facs[:, :])

            uT_sb = cpool.tile([K, T_SEQ], f32)
            nc.sync.dma_start(uT_sb[:, :], uT[:, :])
            spk = cpool.tile([K, T_SEQ], mmdt)
            nc.vector.tensor_scalar(out=spk[:, :], in0=uT_sb[:, :],
                                    scalar1=P_SPIKE, scalar2=None, op0=lt)

            # --- main loop over W chunks ---
            rep_ctx = (tc.For_i(0, reps, 1) if reps > 1
                       else contextlib.nullcontext())
            dmaonly_src = None
            if mode == "dmaonly":
                dmaonly_src = cpool.tile(
                    [P, TT * DMA_GROUP * CHUNK_ROWS * R], odt)
                nc.gpsimd.memset(dmaonly_src[:, :], 1.0)

            with rep_ctx:
                _chunk_loop(nc, tc, t_glob, ntab, spk, rrel_sb, colf_sb,
                            wts_sb, facs_sb, bpool, wpool, opool, pswp, psop,
                            y, mybir, mode, dmaonly_src, mmdt, odt)

    nc.compile()
    return nc


def _chunk_loop(nc, tc, t_glob, ntab, spk, rrel_sb, colf_sb, wts_sb, facs_sb,
                bpool, wpool, opool, pswp, psop, y, mybir, mode="full",
                dmaonly_src=None, mmdt=None, odt=None):
    f32 = mybir.dt.float32
    if mmdt is None:
        mmdt = f32
    if odt is None:
        odt = f32
    eq = mybir.AluOpType.is_equal
    mul = mybir.AluOpType.mult
    CW = CHUNK_ROWS * R          # 510: per-chunk output columns
    G = DMA_GROUP                # chunks per output DMA (long contiguous runs)
    PAIR = G * CW                # osb holds G chunks per t-tile
    osb = None
    for c in range(N_CHUNKS):
        n_c = CHUNK_ROWS if c < N_CHUNKS - 1 else LAST_ROWS
        wdt = R * n_c
        cc = c % G

        if cc == 0:
            pair_c0 = c * CW
            last_in_grp = min(c + G - 1, N_CHUNKS - 1)
            pair_wdt = (last_in_grp - c) * CW + R * (
                CHUNK_ROWS if last_in_grp < N_CHUNKS - 1 else LAST_ROWS)
            dma_eng = nc.sync if (c // G) % 2 == 0 else nc.scalar
            # dst dims ordered (row, tt, q) to match SBUF src iteration order
            y_ap = y.ap().rearrange(
                "(tt row) q -> tt row q",
                tt=TT).transpose([1, 0, 2])[:, :, pair_c0:pair_c0 + pair_wdt]

        if mode == "dmaonly":
            if cc == G - 1 or c == N_CHUNKS - 1:
                src = dmaonly_src[0:T_TILE, :].rearrange(
                    "p (tt q) -> p tt q", tt=TT)[:, :, 0:pair_wdt]
                dma_eng.dma_start(y_ap, src)
            continue

        wsb = wpool.tile([P, CW], mmdt)
        if mode == "noscatter":
            nc.vector.memset(wsb[0:K, 0:wdt], 0.5)
        else:
            psw = pswp.tile([P, CW], f32)
            for i in range(t_glob):
                ti = c * t_glob + i
                lhs_t = bpool.tile([P, K], mmdt, tag="lhsT")
                rhs_t = bpool.tile([P, CW], mmdt, tag="rhs")
                if mode == "nobuild":
                    nc.vector.memset(lhs_t[:, :], 0.5)
                    nc.vector.memset(rhs_t[:, 0:wdt], 0.5)
                else:
                    nc.vector.tensor_scalar(
                        out=lhs_t[:, :], in0=ntab[:, 0:K],
                        scalar1=colf_sb[:, ti:ti + 1],
                        scalar2=wts_sb[:, ti:ti + 1],
                        op0=eq, op1=mul)
                    if i == 0:
                        # offload one rhs build per chunk to the otherwise
                        # idle GPSIMD engine (mask on DVE, multiply on GPS)
                        mask_t = bpool.tile([P, CHUNK_ROWS], f32, tag="mask")
                        nc.vector.tensor_scalar(
                            out=mask_t[:, 0:n_c], in0=ntab[:, 0:n_c],
                            scalar1=rrel_sb[:, ti:ti + 1],
                            scalar2=None, op0=eq)
                        nc.gpsimd.tensor_tensor(
                            out=rhs_t[:, 0:wdt].rearrange(
                                "p (r n) -> p r n", r=R),
                            in0=mask_t[:, 0:n_c].unsqueeze(1)
                                .broadcast_to([P, R, n_c]),
                            in1=facs_sb[:, ti * R:(ti + 1) * R]
                                .unsqueeze(2).broadcast_to([P, R, n_c]),
                            op=mul)
                    else:
                        for r in range(R):
                            nc.vector.tensor_scalar(
                                out=rhs_t[:, r * n_c:(r + 1) * n_c],
                                in0=ntab[:, 0:n_c],
                                scalar1=rrel_sb[:, ti:ti + 1],
                                scalar2=facs_sb[:, ti * R + r:ti * R + r + 1],
                                op0=eq, op1=mul)
                nc.tensor.matmul(psw[0:K, 0:wdt], lhsT=lhs_t[:, :],
                                 rhs=rhs_t[:, 0:wdt],
                                 start=(i == 0), stop=(i == t_glob - 1))

            # psum (k, (r, n)) -> sbuf (k, (n, r))
            in_ap = psw[0:K, 0:wdt].rearrange(
                "k (r n) -> k r n", r=R).transpose([0, 2, 1])
            out_ap = wsb[0:K, 0:wdt].rearrange("k (n r) -> k n r", r=R)
            nc.scalar.copy(out=out_ap, in_=in_ap)

        if cc == 0:
            osb = opool.tile([P, TT * PAIR], odt)
        for tt in range(TT):
            pso = psop.tile([P, CW], f32)
            nc.tensor.matmul(
                pso[0:T_TILE, 0:wdt],
                lhsT=spk[:, tt * T_TILE:(tt + 1) * T_TILE],
                rhs=wsb[0:K, 0:wdt], start=True, stop=True)
            dst = osb[0:T_TILE, tt * PAIR + cc * CW:tt * PAIR + cc * CW + wdt]
            nc.scalar.copy(out=dst, in_=pso[0:T_TILE, 0:wdt])
        if mode != "nodma" and (cc == G - 1 or c == N_CHUNKS - 1):
            src = osb[0:T_TILE, :].rearrange(
                "p (tt q) -> p tt q", tt=TT)[:, :, 0:pair_wdt]
            dma_eng.dma_start(y_ap, src)


def _pack_inputs(u, rows, cols, weights, weights_factors):
    """Host-side sharding prep: bucket COO edges by (core, chunk) into
    128-slot tiles. Returns (t_glob, per-core in_maps list)."""
    u = np.asarray(u, np.float32)
    rows = np.asarray(rows, np.int64)
    cols = np.asarray(cols, np.int64)
    weights = np.asarray(weights, np.float32)
    weights_factors = np.asarray(weights_factors, np.float32)
    nnz = rows.shape[0]

    core = rows // N_SHARD
    nloc = rows - core * N_SHARD
    chunk = nloc // CHUNK_ROWS
    rrel = (nloc - chunk * CHUNK_ROWS).astype(np.float32)

    n_buckets = N_CORES * N_CHUNKS
    key = (core * N_CHUNKS + chunk).astype(np.int64)
    order = np.argsort(key, kind="stable")
    counts = np.bincount(key, minlength=n_buckets)
    t_glob = max(1, int(-(-counts.max() // P)))
    S = t_glob * P

    offsets = np.zeros(n_buckets, np.int64)
    np.cumsum(counts[:-1], out=offsets[1:])
    rank = np.arange(nnz, dtype=np.int64) - offsets[key[order]]
    slot = key[order] * S + rank

    tot = n_buckets * S
    rrel_s = np.full(tot, -1.0, np.float32)
    colf_s = np.zeros(tot, np.float32)
    wts_s = np.zeros(tot, np.float32)
    facs_s = np.zeros((tot, R), np.float32)
    rrel_s[slot] = rrel[order]
    colf_s[slot] = cols[order].astype(np.float32)
    wts_s[slot] = weights[order]
    facs_s[slot] = weights_factors[order]

    uT = np.ascontiguousarray(u.reshape(T_SEQ, K).T)

    n_tiles = N_CHUNKS * t_glob
    per_core = S * N_CHUNKS
    in_maps = []
    for k in range(N_CORES):
        sl = slice(k * per_core, (k + 1) * per_core)
        rr = np.ascontiguousarray(rrel_s[sl].reshape(n_tiles, P).T)
        cf = np.ascontiguousarray(colf_s[sl].reshape(n_tiles, P).T)
        wt = np.ascontiguousarray(wts_s[sl].reshape(n_tiles, P).T)
        fa = np.ascontiguousarray(
            facs_s[sl].reshape(n_tiles, P, R).transpose(1, 0, 2).reshape(
                P, n_tiles * R))
        in_maps.append({"uT": uT, "rrel": rr, "colf": cf, "wts": wt,
                        "facs": fa})
    return t_glob, in_maps


def kernel(u, rows, cols, weights, weights_factors):
    from concourse.bass_utils import run_bass_kernel_spmd

    t_glob, in_maps = _pack_inputs(u, rows, cols, weights, weights_factors)

    nc = _CACHE.get(t_glob)
    if nc is None:
        nc = _build_nc(t_glob)
        _CACHE[t_glob] = nc

    res = run_bass_kernel_spmd(nc, in_maps, core_ids=list(range(N_CORES)))

    out = np.empty((T_SEQ, N_NEURONS * R), np.float32)
    for k in range(N_CORES):
        out[:, k * N_SHARD * R:(k + 1) * N_SHARD * R] = (
            res.results[k]["y"].astype(np.float32))
    return out.reshape(1, T_SEQ, N_NEURONS * R)


if __name__ == "__main__":
    rng = np.random.default_rng(0)
    u = rng.random((1, T_SEQ, K), dtype=np.float32)
    rows = rng.integers(0, N_NEURONS, 20000).astype(np.int64)
    cols = rng.integers(0, K, 20000).astype(np.int64)
    weights = rng.standard_normal(20000).astype(np.float32)
    wf = rng.random((20000, R), dtype=np.float32)
    out = kernel(u=u, rows=rows, cols=cols, weights=weights,
                 weights_factors=wf)
    print("out", out.shape, out.dtype, float(np.abs(out).max()))

